# revision 1
# baseline (speedup 1.0000x reference)
"""Cross-attention (GQA) Trainium2 Bass kernel.

Problem: B=2, Tq=Tkv=2048, D_MODEL=1024, 16 query heads / 4 kv heads,
head_dim=64.  Sharded over 8 NeuronCores as batch(2) x kv-group(4); each
core computes 4 query heads + its single kv head and a partial output
projection (Wo row-split by head group); partials are summed on host.

On-chip dataflow keeps activations "transposed" (feature dim on SBUF
partitions) end-to-end so that scores, softmax and P@V need no on-chip
transposes of large tensors:

  A: qT[e,t] = WqT.T @ xqT,  kvT = WkvT.T @ xcT        (fp32r, N=512)
     v[tk,dv] via PE-transpose of vT tiles
  B: ST[tk,tq] = kT.T @ qT_h ; two heads packed in the PE array via
     row-groups (K=64 each, h_even rows 0-63, h_odd rows 64-127)
  C: P = exp(ST/8)  on ScalarE, PSUM->SBUF, 1024-wide instructions
  D: outT'[dv+sum,tq] = [v|1].T @ P ; the ones-column matmul is
     col-packed into a spare PE column-group => denominators come out
     of the same pass.  h_odd heads are placed at partitions 64..127.
  E: yT += WoT_pair.T @ outT_norm (K=128: two heads stacked)
"""

import os
import sys

import numpy as np

for _p in ("/opt/trn_rl_repo",):
    if _p not in sys.path and os.path.isdir(_p):
        sys.path.insert(0, _p)

import concourse.bass as bass
import concourse.bacc as bacc
import concourse.mybir as mybir
from concourse.tile import TileContext

# ---------------------------------------------------------------- problem dims
B = 2
TQ = 2048
TKV = 2048
D_MODEL = 1024
N_HEADS = 16
N_KV_HEADS = 4
HEAD_DIM = 64
N_CORES = 8
GROUPS = N_KV_HEADS  # kv groups = 4
HEADS_PER_DEV = N_HEADS // GROUPS  # 4
DQ = HEADS_PER_DEV * HEAD_DIM  # 256
DKV = 2 * HEAD_DIM  # 128 (k rows + v rows stacked)
SCALE = 1.0 / float(np.sqrt(HEAD_DIM))

P = 128
FREE = 512  # matmul moving-operand chunk
BLK = 1024  # tq block width (exp instruction width)

F32 = mybir.dt.float32
F32R = mybir.dt.float32r
F16 = mybir.dt.float16


def build_bass():
    nc = bacc.Bacc()

    xq = nc.declare_dram_parameter("xqT", [D_MODEL, TQ], F16, isOutput=False)
    xc = nc.declare_dram_parameter("xcT", [D_MODEL, TKV], F16, isOutput=False)
    wq = nc.declare_dram_parameter("wqT", [D_MODEL, DQ], F16, isOutput=False)
    wkv = nc.declare_dram_parameter("wkvT", [D_MODEL, DKV], F16, isOutput=False)
    wo = nc.declare_dram_parameter("woT", [DQ, D_MODEL], F16, isOutput=False)
    cid = nc.declare_dram_parameter("cid", [P, P + 64], F16, isOutput=False)
    yt = nc.declare_dram_parameter("yT", [D_MODEL, TQ], F32, isOutput=True)

    DT = D_MODEL // P  # 8 d-tiles
    ET = DQ // P  # 2 e-tiles (query head pairs)
    NCH = TQ // FREE  # 4 chunks of 512
    NTK = TKV // P  # 16 tk tiles
    NBLK = TQ // BLK  # 2 tq blocks
    JPB = BLK // FREE  # 2 free-chunks per block
    MT = D_MODEL // P  # 8 output m-tiles

    with TileContext(nc) as tc:
        with (
            tc.tile_pool(name="consts", bufs=1) as consts,
            tc.tile_pool(name="xch", bufs=3) as xpool,
            tc.tile_pool(name="pt", bufs=6) as ptpool,
            tc.tile_pool(name="nrm", bufs=2) as nrmpool,
            tc.tile_pool(name="yout", bufs=3) as ypool,
            tc.tile_pool(name="psA", bufs=2, space="PSUM") as psA,
            tc.tile_pool(name="psB", bufs=2, space="PSUM") as psB,
        ):
            # ---------------- constants / persistent tiles
            ident = consts.tile([P, P + 64], F16, tag="ident")
            nc.sync.dma_start(ident, cid[:])
            ones = ident[:, P : P + 64]

            wq_sb = consts.tile([P, DT, DQ], F16, tag="wq")
            nc.sync.dma_start(wq_sb, wq.rearrange("(i p) e -> p i e", p=P))
            wkv_sb = consts.tile([P, DT, DKV], F16, tag="wkv")
            nc.sync.dma_start(wkv_sb, wkv.rearrange("(i p) e -> p i e", p=P))
            wo_sb = consts.tile([P, ET, D_MODEL], F16, tag="wo")
            nc.sync.dma_start(wo_sb, wo.rearrange("(i p) m -> p i m", p=P))

            qt = consts.tile([P, ET, TQ], F16, tag="qt")  # qT: heads 2/tile
            kv = consts.tile([P, TKV], F16, tag="kv")  # rows 0-63 kT, 64-127 vT
            k2 = consts.tile([P, TKV], F16, tag="k2")  # rows 64-127 = kT copy
            vp = consts.tile([P, NTK, P], F16, tag="vp")  # [v | ones]
            vp2 = consts.tile([P, NTK, P], F16, tag="vp2")  # [ones | v]
            outs = consts.tile([P, ET, TQ], F16, tag="outs")  # normalized outT

            # ---------------- stage A: projections (weights stationary)
            # kv first (every BCD iteration needs the full kT/vT), then q
            for c in range(NCH):
                cs = slice(c * FREE, (c + 1) * FREE)
                xc_t = xpool.tile([P, DT, FREE], F16, tag="xch")
                nc.sync.dma_start(
                    xc_t, xc.rearrange("(i p) t -> p i t", p=P)[:, :, cs]
                )
                pkv = psB.tile([P, FREE], F32, tag="psB")
                for i in range(DT):
                    nc.tensor.matmul(
                        pkv,
                        (wkv_sb[:, i, :]),
                        (xc_t[:, i, :]),
                        start=(i == 0),
                        stop=(i == DT - 1),
                    )
                nc.vector.tensor_copy(kv[:, cs], pkv)
                # duplicate kT rows into partitions 64..127 for row-packing
                nc.sync.dma_start(k2[HEAD_DIM : 2 * HEAD_DIM, cs], kv[:HEAD_DIM, cs])

            def emit_q_chunk(c):
                cs = slice(c * FREE, (c + 1) * FREE)
                xq_t = xpool.tile([P, DT, FREE], F16, tag="xch", name="xq_t")
                nc.sync.dma_start(
                    xq_t, xq.rearrange("(i p) t -> p i t", p=P)[:, :, cs]
                )
                for e in range(ET):
                    pq = psA.tile([P, FREE], F32, tag="psA", name="pq")
                    for i in range(DT):
                        nc.tensor.matmul(
                            pq,
                            (wq_sb[:, i, e * P : (e + 1) * P]),
                            (xq_t[:, i, :]),
                            start=(i == 0),
                            stop=(i == DT - 1),
                        )
                    nc.vector.tensor_copy(qt[:, e, cs], pq)

            for _c in range(min(2, NCH)):
                emit_q_chunk(_c)

            # v' tiles: PE-transpose vT[64, tk*128 ..] -> [128, 64], then
            # build [v | ones] (for even heads) and [ones | v] (odd heads).
            # The all-ones half makes the same matmul emit the softmax
            # denominators, replicated across 64 partitions.
            for t in range(NTK):
                ts_ = slice(t * P, (t + 1) * P)
                pv = psB.tile([P, HEAD_DIM], F16, tag="psB")
                nc.tensor.transpose(
                    pv, kv[HEAD_DIM : 2 * HEAD_DIM, ts_], ident[HEAD_DIM:, HEAD_DIM:P]
                )
                nc.vector.tensor_copy(vp[:, t, :HEAD_DIM], pv)
                nc.vector.tensor_copy(vp2[:, t, HEAD_DIM:], pv)
                nc.vector.tensor_copy(vp[:, t, HEAD_DIM:], ones)
                nc.vector.tensor_copy(vp2[:, t, :HEAD_DIM], ones)

            # -------- stage E chunk emitter (interleaved into BCD stream)
            def emit_out_chunk(c):
                cs = slice(c * FREE, (c + 1) * FREE)
                for m in range(MT):
                    ms = slice(m * P, (m + 1) * P)
                    py = psA.tile([P, FREE], F32, tag="psA", name="py")
                    for ee in range(ET):
                        nc.tensor.matmul(
                            py,
                            (wo_sb[:, ee, ms]),
                            (outs[:, ee, cs]),
                            start=(ee == 0),
                            stop=(ee == ET - 1),
                        )
                    yo = ypool.tile([P, FREE], F32, tag="yout", name="yo")
                    nc.vector.tensor_copy(yo, py)
                    nc.sync.dma_start(yt[ms, cs], yo)

            # ---------------- stages B/C/D: attention per head-pair
            first_bcd = True
            for blk in range(NBLK):
                for e in range(ET):  # head pair (h_even=2e, h_odd=2e+1)
                    bs = slice(blk * BLK, (blk + 1) * BLK)
                    pd = [
                        psB.tile([P, BLK], F32, tag="psB", name=f"pd{_h}")
                        for _h in range(2)
                    ]  # D accumulators: [0]=h_even rows 0-64, [1]=h_odd
                    for t in range(NTK):
                        ts_ = slice(t * P, (t + 1) * P)
                        pb = [
                            psA.tile([P, BLK], F32, tag="psA", name=f"pb{_h}")
                            for _h in range(2)
                        ]
                        for j in range(JPB):
                            js = slice(blk * BLK + j * FREE, blk * BLK + (j + 1) * FREE)
                            jo = slice(j * FREE, (j + 1) * FREE)
                            # scores, 2 heads row-packed (K=64 each)
                            nc.tensor.matmul(
                                pb[0][:, jo],
                                (kv[:HEAD_DIM, ts_]),
                                (qt[:HEAD_DIM, e, js]),
                            )
                            nc.tensor.matmul(
                                pb[1][:, jo],
                                (k2[HEAD_DIM:, ts_]),
                                (qt[HEAD_DIM:, e, js]),
                            )
                        for h in range(2):
                            pt = ptpool.tile([P, BLK], F16, tag="pt")
                            nc.scalar.activation(
                                pt,
                                pb[h],
                                mybir.ActivationFunctionType.Exp,
                                bias=0.0,
                                scale=SCALE,
                            )
                            # M=128 stationary [v|ones] / [ones|v]: one
                            # matmul per head yields out_h in its 64-row
                            # half and the softmax denominators (replicated
                            # x64) in the other half.  dst base stays 0
                            # (fp32r matmuls cannot target offset psum
                            # partitions).
                            vo = vp if h == 0 else vp2
                            for j in range(JPB):
                                jo = slice(j * FREE, (j + 1) * FREE)
                                nc.tensor.matmul(
                                    pd[h][:, jo],
                                    vo[:, t, :],
                                    pt[:, jo],
                                    start=(t == 0),
                                    stop=(t == NTK - 1),
                                    skip_group_check=True,
                                )
                    if first_bcd:
                        first_bcd = False
                        for _c in range(2, NCH):
                            emit_q_chunk(_c)
                    # spill raw accumulators to SBUF immediately (~1.2us)
                    # so the PSUM slots free up and the PE never stalls;
                    # the normalize chain below runs off the critical path.
                    for h in range(2):
                        raw = nrmpool.tile([P, BLK], F32, tag=f"raw{h}")
                        nc.vector.tensor_copy(raw, pd[h])
                        lo = slice(0, 64) if h == 0 else slice(64, 128)
                        hi = slice(64, 128) if h == 0 else slice(0, 64)
                        rec = nrmpool.tile([P, BLK], F32, tag="rec")
                        rec2 = nrmpool.tile([P, BLK], F32, tag="rec2")
                        nc.vector.reciprocal(rec[hi, :], raw[hi, :])
                        nc.sync.dma_start(rec2[lo, :], rec[hi, :])
                        nc.vector.tensor_mul(
                            outs[lo, e, bs], raw[lo, :], rec2[lo, :]
                        )
                    if e == ET - 1:
                        for _c in range(blk * (BLK // FREE), (blk + 1) * (BLK // FREE)):
                            emit_out_chunk(_c)


    nc.finalize()  # Bacc: runs wait-splitting/reg-alloc passes
    return nc


_NC_CACHE = None


def _get_nc():
    global _NC_CACHE
    if _NC_CACHE is None:
        _NC_CACHE = build_bass()
    return _NC_CACHE


def _cid():
    c = np.zeros((P, P + 64), dtype=np.float16)
    c[:, :P] = np.eye(P, dtype=np.float32)
    c[:, P:] = 1.0
    return c


def shard_inputs(query, context, Wq, Wk, Wv, Wo):
    """host-side sharding: 8 cores = batch(2) x kv-group(4)"""
    in_maps = []
    xqT = [np.ascontiguousarray(query[b].T).astype(np.float16) for b in range(B)]
    xcT = [np.ascontiguousarray(context[b].T).astype(np.float16) for b in range(B)]
    for core in range(N_CORES):
        b, g = divmod(core, GROUPS)
        wqT = np.ascontiguousarray(Wq[g * DQ : (g + 1) * DQ, :].T).astype(np.float16)
        wkvT = np.ascontiguousarray(
            np.concatenate(
                [
                    Wk[g * HEAD_DIM : (g + 1) * HEAD_DIM, :],
                    Wv[g * HEAD_DIM : (g + 1) * HEAD_DIM, :],
                ],
                axis=0,
            ).T
        ).astype(np.float16)
        woT = np.ascontiguousarray(Wo[:, g * DQ : (g + 1) * DQ].T).astype(np.float16)
        in_maps.append(
            {
                "xqT": xqT[b],
                "xcT": xcT[b],
                "wqT": wqT,
                "wkvT": wkvT,
                "woT": woT,
                "cid": _cid(),
            }
        )
    return in_maps


def kernel(query, context, Wq, Wk, Wv, Wo, _want_profile=False):
    from concourse.bass_utils import run_bass_kernel_spmd

    nc = _get_nc()
    in_maps = shard_inputs(query, context, Wq, Wk, Wv, Wo)
    res = run_bass_kernel_spmd(
        nc, in_maps, core_ids=list(range(N_CORES)), trace=_want_profile
    )
    out = np.zeros((B, TQ, D_MODEL), dtype=np.float32)
    for core in range(N_CORES):
        b = core // GROUPS
        out[b] += res.results[core]["yT"].T
    if _want_profile:
        return out, res
    return out



# revision 30
# speedup vs baseline: 1.4461x; 1.4461x over previous
"""Cross-attention (GQA) Trainium2 Bass kernel, v2.

Problem: B=2, Tq=Tkv=2048, D_MODEL=1024, 16 query heads / 4 kv heads,
head_dim=64.  Sharded over 8 NeuronCores as batch(2) x kv-group(4); each
core computes 4 query heads + its single kv head and a partial output
projection (Wo row-split by head group); partials are summed on host.

v2 changes vs v1 (367945 ns):
  * PE kept continuously busy so it ramps to its 2.4 GHz p-state (v1 ran
    at the 1.2 GHz mid p-state throughout: every matmul was 426ns/512col).
  * exp split across two engines: ACT computes true exp into fp8e4 tiles
    (feeding DoubleRow P@V matmuls, 2x contraction per pass); DVE computes
    a Schraudolph bit-trick exp (i16 = a*s + b, bitcast as f16, ~3% max
    elementwise error that washes out in the softmax average).
  * P@V for ACT-produced tiles runs in fp8 DoubleRow: two tk-tiles per
    pass, halving PE columns for those tiles.
  * Softmax denominators come from ones-columns packed into the P@V
    stationary (v1 trick, kept); reciprocal via reciprocal_approx_fast
    (v1 used the 5x slower exact InstReciprocal).
  * single-head iterations (blk x head) with a software-pipelined
    B->exp->D schedule; PSUM: 3x2 banks score pool + 1x2 banks accum.
"""

import os
import sys

import numpy as np

for _p in ("/opt/trn_rl_repo",):
    if _p not in sys.path and os.path.isdir(_p):
        sys.path.insert(0, _p)

import concourse.bass as bass
import concourse.bacc as bacc
import concourse.mybir as mybir
from concourse.tile import TileContext

# ---------------------------------------------------------------- problem dims
B = 2
TQ = 2048
TKV = 2048
D_MODEL = 1024
N_HEADS = 16
N_KV_HEADS = 4
HEAD_DIM = 64
N_CORES = 8
GROUPS = N_KV_HEADS  # kv groups = 4
HEADS_PER_DEV = N_HEADS // GROUPS  # 4
DQ = HEADS_PER_DEV * HEAD_DIM  # 256
DKV = 2 * HEAD_DIM  # 128 (k rows + v rows stacked)
SCALE = 1.0 / float(np.sqrt(HEAD_DIM))

P = 128
FREE = 512  # matmul moving-operand chunk / psum bank width
BLK = 1024  # tq block width

F32 = mybir.dt.float32
F16 = mybir.dt.float16
F8 = mybir.dt.float8e4
I16 = mybir.dt.int16

DT = D_MODEL // P  # 8 d-tiles
ET = DQ // P  # 2 e-tiles (query head pairs)
NCH = TQ // FREE  # 4 chunks of 512
NTK = TKV // P  # 16 tk tiles
NBLK = TQ // BLK  # 2 tq blocks
JPB = BLK // FREE  # 2 free-chunks per block
MT = D_MODEL // P  # 8 output m-tiles

# exp work split: ACT handles PAIRS (true exp -> fp8, DoubleRow P@V),
# DVE handles SGL (bit-trick exp -> i16-as-f16, fp16 P@V).
PAIRS = [(0, 1), (3, 4), (6, 7), (9, 10), (12, 13)]
SGL = [2, 5, 8, 11, 14, 15]
NPAIR = len(PAIRS)
NSGL = len(SGL)
# Both exp paths compute e^(s/8 - SHIFT); the common factor cancels in the
# softmax normalization.  SHIFT keeps the fp8e4 path below the e4m3 max of
# 448.  Observed s/8 range on this data (jax axon-platform RNG!): [-9.0, 9.2]
# -> max fp8 value e^(9.2-3.75) = 217.
SHIFT = 3.75
# Scores are computed with a constant offset row: s' = s + C_DEV (stationary
# row 64 = C, moving row 64 = 1).  The DVE bit-exp is then a single fused
# (mult, max): i16 = max(AEXP*s', 0) -- the max clamps any very negative
# score to P=+0.0 instead of a negative-i16 NaN bitcast.  C absorbs the
# 2^10-exponent bias (15*1024), the Schraudolph centering (-44.5) and SHIFT.
AEXP = 1024.0 * 1.4426950408889634 / 8.0
_LOG2E = 1.4426950408889634
C_OFFSET = (15360.0 - 44.5 - SHIFT * 1024.0 * _LOG2E) / AEXP
C_DEV = float(np.float16(C_OFFSET))  # value actually contracted on device
# DVE path scale: P_DVE = e^(s/8) * 2^(AEXP*C_DEV/1024 - 15).  ACT bias makes
# the true-exp path match: P_ACT = e^(s'/8 + bias) = e^(s/8) * same-scale.
ACT_BIAS = float(np.log(2.0) * (AEXP * C_DEV / 1024.0 - 15.0) - C_DEV / 8.0)

_T2UNIT = {}  # tile -> ('P', pair_idx, plane) | ('S', sgl_idx)
for _k, (_a, _b) in enumerate(PAIRS):
    _T2UNIT[_a] = ("P", _k, 0)
    _T2UNIT[_b] = ("P", _k, 1)
for _k, _t in enumerate(SGL):
    _T2UNIT[_t] = ("S", _k)

# D-unit emission schedule: emit after E(last_tile+2) (cap 15)
_DPOS = {}  # t_emit -> list of units ('P', k) or ('S', k)
for _k, (_a, _b) in enumerate(PAIRS):
    _DPOS.setdefault(min(_b + 2, NTK - 1), []).append(("P", _k))
for _k, _t in enumerate(SGL):
    _DPOS.setdefault(min(_t + 2, NTK - 1), []).append(("S", _k))
_NUNITS = NPAIR + NSGL


def build_bass(use_fp8=True, use_bitexp=True, debug=False):
    nc = bacc.Bacc()

    xq = nc.declare_dram_parameter("xqT", [D_MODEL, TQ], F16, isOutput=False)
    xc = nc.declare_dram_parameter("xcT", [D_MODEL, TKV], F16, isOutput=False)
    wq = nc.declare_dram_parameter("wqT", [D_MODEL, DQ], F16, isOutput=False)
    wkv = nc.declare_dram_parameter("wkvT", [D_MODEL, DKV], F16, isOutput=False)
    wo = nc.declare_dram_parameter("woT", [DQ, D_MODEL], F16, isOutput=False)
    cid = nc.declare_dram_parameter("cid", [P, P + 64], F16, isOutput=False)
    yt = nc.declare_dram_parameter("yT", [D_MODEL, TQ], F32, isOutput=True)

    with TileContext(nc) as tc:
        with (
            tc.tile_pool(name="consts", bufs=1) as consts,
            tc.tile_pool(name="xch", bufs=8) as xpool,
            tc.tile_pool(name="pt8", bufs=2) as pt8pool,
            tc.tile_pool(name="pt16", bufs=2) as pt16pool,
            tc.tile_pool(name="nrm", bufs=2) as nrmpool,
            tc.tile_pool(name="yout", bufs=3) as ypool,
            tc.tile_pool(name="psB", bufs=3, space="PSUM") as psB,
            tc.tile_pool(name="psD", bufs=1, space="PSUM") as psD,
        ):
            # ---------------- constants / persistent tiles
            ident = consts.tile([P, P + 64], F16, tag="ident")
            nc.sync.dma_start(ident, cid[:])

            wq_sb = consts.tile([P, DT, DQ], F16, tag="wq")
            nc.sync.dma_start(wq_sb, wq.rearrange("(i p) e -> p i e", p=P))
            wkv_sb = consts.tile([P, DT, DKV], F16, tag="wkv")
            nc.sync.dma_start(wkv_sb, wkv.rearrange("(i p) e -> p i e", p=P))
            wo_sb = consts.tile([P, ET, D_MODEL], F16, tag="wo")
            nc.sync.dma_start(wo_sb, wo.rearrange("(i p) m -> p i m", p=P))

            # input staging: all chunks resident
            xc_t = consts.tile([P, NCH, DT, FREE], F16, tag="xc")
            xq_t = consts.tile([P, NCH, DT, FREE], F16, tag="xq")
            xc_r = xc.rearrange("(i p) t -> p i t", p=P)
            xq_r = xq.rearrange("(i p) t -> p i t", p=P)

            def load_chunk(dst, src, c):
                cs = slice(c * FREE, (c + 1) * FREE)
                for i in range(DT):
                    nc.sync.dma_start(dst[:, c, i, :], src[:, i, cs])

            # DMA priority order: xc0, xq0, xq1, xc1-3, xq2, xq3
            load_chunk(xc_t, xc_r, 0)
            load_chunk(xq_t, xq_r, 0)
            load_chunk(xq_t, xq_r, 1)
            for c in range(1, NCH):
                load_chunk(xc_t, xc_r, c)
            load_chunk(xq_t, xq_r, 2)
            load_chunk(xq_t, xq_r, 3)

            qt = consts.tile([P, ET, TQ], F16, tag="qt")  # qT staging: 2 heads/tile
            kv = consts.tile([P, TKV], F16, tag="kv")  # rows 0-63 kT, 64-127 vT
            # B operands with the offset row: kc = [kT; C], qt65 = [q_h; 1]
            kc = consts.tile([P, TKV], F16, tag="kc")
            qt65 = consts.tile([P, HEADS_PER_DEV, TQ], F16, tag="qt65")
            # P@V stationaries. A: [v|ones] (even heads), B: [ones|v] (odd)
            vp8A = consts.tile([P, NPAIR, 2, P], F8, tag="vp8A")
            vp8B = consts.tile([P, NPAIR, 2, P], F8, tag="vp8B")
            vp16A = consts.tile([P, NTK, P], F16, tag="vp16A")
            vp16B = consts.tile([P, NTK, P], F16, tag="vp16B")
            outs = consts.tile([P, ET, TQ], F16, tag="outs")  # normalized outT

            nc.vector.memset(vp8A[:, :, :, HEAD_DIM:], 1.0)
            nc.vector.memset(vp8B[:, :, :, :HEAD_DIM], 1.0)
            nc.vector.memset(vp16A[:, :, HEAD_DIM:], 1.0)
            nc.vector.memset(vp16B[:, :, :HEAD_DIM], 1.0)

            bshift = consts.tile([P, 1], F32, tag="bshift")
            nc.vector.memset(bshift, ACT_BIAS)
            nc.vector.memset(kc[HEAD_DIM : HEAD_DIM + 1, :], C_DEV)
            nc.vector.memset(qt65[HEAD_DIM : HEAD_DIM + 1, :, :], 1.0)

            # ---------------- PE warmup: ramp the p-state while DMAs land
            for w in range(3):
                pwarm = psB.tile([P, 192], F32, tag="ps", name=f"warm{w}")
                for i in range(8):
                    nc.tensor.matmul(
                        pwarm, ident[:, :P], ident[:], start=(i == 0), stop=(i == 7)
                    )

            # ---------------- stage A helpers
            def emit_kv_chunk(c):
                cs = slice(c * FREE, (c + 1) * FREE)
                pkv = psB.tile([P, FREE], F32, tag="ps", name=f"pkv{c}")
                for i in range(DT):
                    nc.tensor.matmul(
                        pkv,
                        wkv_sb[:, i, :],
                        xc_t[:, c, i, :],
                        start=(i == 0),
                        stop=(i == DT - 1),
                    )
                nc.vector.tensor_copy(kv[:, cs], pkv)
                nc.sync.dma_start(kc[:HEAD_DIM, cs], kv[:HEAD_DIM, cs])

            def emit_q_chunk(c, e):
                cs = slice(c * FREE, (c + 1) * FREE)
                pq = psB.tile([P, FREE], F32, tag="ps", name=f"pq{c}_{e}")
                for i in range(DT):
                    nc.tensor.matmul(
                        pq,
                        wq_sb[:, i, e * P : (e + 1) * P],
                        xq_t[:, c, i, :],
                        start=(i == 0),
                        stop=(i == DT - 1),
                    )
                nc.vector.tensor_copy(qt[:, e, cs], pq)
                nc.sync.dma_start(qt65[:HEAD_DIM, 2 * e, cs], qt[:HEAD_DIM, e, cs])
                nc.sync.dma_start(
                    qt65[:HEAD_DIM, 2 * e + 1, cs], qt[HEAD_DIM:, e, cs]
                )

            def emit_v_tile(t):
                ts_ = slice(t * P, (t + 1) * P)
                pv = psB.tile([P, HEAD_DIM], F16, tag="ps", name=f"pv{t}")
                nc.tensor.transpose(
                    pv, kv[HEAD_DIM : 2 * HEAD_DIM, ts_], ident[HEAD_DIM:, HEAD_DIM:P]
                )
                u = _T2UNIT[t]
                if u[0] == "P" and use_fp8:
                    _, k, pl = u
                    nc.vector.tensor_copy(vp8A[:, k, pl, :HEAD_DIM], pv)
                    nc.vector.tensor_copy(vp8B[:, k, pl, HEAD_DIM:], pv)
                if u[0] == "S" or not use_fp8:
                    nc.vector.tensor_copy(vp16A[:, t, :HEAD_DIM], pv)
                    nc.vector.tensor_copy(vp16B[:, t, HEAD_DIM:], pv)

            # ---------------- stage E (output projection) per 512-chunk
            def emit_out_chunk(c):
                cs = slice(c * FREE, (c + 1) * FREE)
                for m in range(MT):
                    ms = slice(m * P, (m + 1) * P)
                    py = psB.tile([P, FREE], F32, tag="ps", name=f"py{c}_{m}")
                    for ee in range(ET):
                        nc.tensor.matmul(
                            py,
                            wo_sb[:, ee, ms],
                            outs[:, ee, cs],
                            start=(ee == 0),
                            stop=(ee == ET - 1),
                        )
                    yo = ypool.tile([P, FREE], F32, tag="yout", name=f"yo{c}_{m}")
                    nc.vector.tensor_copy(yo, py)
                    nc.sync.dma_start(yt[ms, cs], yo)

            if debug:
                ddnm = nc.declare_dram_parameter(
                    "dbg_dnm", [HEAD_DIM, 8 * BLK], F32, isOutput=True
                )
                drec = nc.declare_dram_parameter(
                    "dbg_rec", [HEAD_DIM, 8 * BLK], F32, isOutput=True
                )
                dpt16 = nc.declare_dram_parameter(
                    "dbg_pt16", [P, NSGL * BLK], I16, isOutput=True
                )
                dpb = nc.declare_dram_parameter(
                    "dbg_pb", [P, NSGL * BLK], F32, isOutput=True
                )

            # ---------------- one (blk, head) iteration of B -> exp -> D
            def emit_iteration(blk, e, hpar, extra=None):
                """extra: optional list of callables interleaved at fixed
                tile steps to fold stage-A tail work into the PE stream."""
                bs = slice(blk * BLK, (blk + 1) * BLK)
                head = 2 * e + hpar
                vp8 = vp8A if hpar == 0 else vp8B
                vp16 = vp16A if hpar == 0 else vp16B

                pd = psD.tile([P, BLK], F32, tag="pd", name=f"pd{blk}_{e}_{hpar}")
                pt8s = {}
                pt16s = {}
                unit_no = [0]

                def emit_d(unit):
                    first = unit_no[0] == 0
                    last = unit_no[0] == _NUNITS - 1
                    unit_no[0] += 1
                    if unit[0] == "P":
                        k = unit[1]
                        src = pt8s.pop(k)
                        for j in range(JPB):
                            jo = slice(j * FREE, (j + 1) * FREE)
                            if use_fp8:
                                nc.tensor.matmul(
                                    pd[:, jo],
                                    vp8[:, k, :, :],
                                    src[:, :, jo],
                                    start=first,
                                    stop=last,
                                    perf_mode=mybir.MatmulPerfMode.DoubleRow,
                                    skip_group_check=True,
                                )
                            else:
                                for pl in range(2):
                                    nc.tensor.matmul(
                                        pd[:, jo],
                                        vp16[:, PAIRS[k][pl], :],
                                        src[:, pl, jo],
                                        start=first and pl == 0,
                                        stop=last and pl == 1,
                                        skip_group_check=True,
                                    )
                    else:
                        k = unit[1]
                        t_ = SGL[k]
                        src = pt16s.pop(k)
                        if use_bitexp:
                            src = src.bitcast(F16)
                        for j in range(JPB):
                            jo = slice(j * FREE, (j + 1) * FREE)
                            nc.tensor.matmul(
                                pd[:, jo],
                                vp16[:, t_, :],
                                src[:, jo],
                                start=first,
                                stop=last,
                                skip_group_check=True,
                            )

                for t in range(NTK):
                    ts_ = slice(t * P, (t + 1) * P)
                    # B: scores (+offset row) for this head, tile t; K=65
                    pb = psB.tile([P, BLK], F32, tag="ps", name=f"pb{t}")
                    for j in range(JPB):
                        js = slice(blk * BLK + j * FREE, blk * BLK + (j + 1) * FREE)
                        jo = slice(j * FREE, (j + 1) * FREE)
                        nc.tensor.matmul(
                            pb[:, jo],
                            kc[: HEAD_DIM + 1, ts_],
                            qt65[: HEAD_DIM + 1, head, js],
                        )
                    # exp
                    u = _T2UNIT[t]
                    if u[0] == "P":
                        _, k, pl = u
                        if pl == 0:
                            pt8s[k] = pt8pool.tile(
                                [P, 2, BLK],
                                F8 if use_fp8 else F16,
                                tag="pt8",
                                name=f"pt8_{k}",
                            )
                        nc.scalar.activation(
                            pt8s[k][:, pl, :],
                            pb,
                            mybir.ActivationFunctionType.Exp,
                            bias=bshift[:, :],
                            scale=SCALE,
                        )
                    else:
                        _, k = u
                        pt16s[k] = pt16pool.tile(
                            [P, BLK],
                            I16 if use_bitexp else F16,
                            tag="pt16",
                            name=f"pt16_{k}",
                        )
                        if use_bitexp:
                            nc.vector.tensor_scalar(
                                pt16s[k],
                                pb,
                                AEXP,
                                0.0,
                                mybir.AluOpType.mult,
                                mybir.AluOpType.max,
                            )
                        else:
                            nc.scalar.activation(
                                pt16s[k],
                                pb,
                                mybir.ActivationFunctionType.Exp,
                                bias=bshift[:, :],
                                scale=SCALE,
                            )
                        if debug and blk == 0 and e == 0 and hpar == 0:
                            if use_bitexp:
                                nc.sync.dma_start(
                                    dpt16.rearrange("p (i t) -> p i t", i=NSGL)[
                                        :, k, :
                                    ],
                                    pt16s[k],
                                )
                            pbc = nrmpool.tile([P, BLK], F32, tag="pbc")
                            nc.vector.tensor_copy(pbc, pb)
                            nc.sync.dma_start(
                                dpb.rearrange("p (i t) -> p i t", i=NSGL)[:, k, :],
                                pbc,
                            )
                    # D units scheduled at this step
                    for unit in _DPOS.get(t, ()):
                        emit_d(unit)
                    if extra and t in extra:
                        extra[t]()

                # normalize: out = num * approx(1/denom).
                # reciprocal_approx_fast is broken on partitions 64-127 (its
                # custom-DVE uops assume base partition 0), so always run it
                # on partitions 0-63, shifting the denominator down first for
                # even heads (denominator lands on PSUM rows 64-127 there).
                rec = nrmpool.tile([P, BLK], F32, tag="rec")
                if hpar == 0:
                    dnm = nrmpool.tile([P, BLK], F32, tag="dnm")
                    dnm2 = nrmpool.tile([P, BLK], F32, tag="dnm2")
                    nc.vector.tensor_copy(dnm[HEAD_DIM:, :], pd[HEAD_DIM:, :])
                    nc.sync.dma_start(dnm2[:HEAD_DIM, :], dnm[HEAD_DIM:, :])
                    nc.vector.reciprocal_approx_fast(
                        rec[:HEAD_DIM, :], dnm2[:HEAD_DIM, :]
                    )
                    nc.vector.tensor_mul(
                        outs[:HEAD_DIM, e, bs], pd[:HEAD_DIM, :], rec[:HEAD_DIM, :]
                    )
                    if debug:
                        it = 4 * blk + 2 * e + hpar
                        nc.sync.dma_start(
                            ddnm.rearrange("p (i t) -> p i t", i=8)[:, it, :],
                            dnm2[:HEAD_DIM, :],
                        )
                        nc.sync.dma_start(
                            drec.rearrange("p (i t) -> p i t", i=8)[:, it, :],
                            rec[:HEAD_DIM, :],
                        )
                else:
                    rec2 = nrmpool.tile([P, BLK], F32, tag="rec2")
                    nc.vector.reciprocal_approx_fast(rec[:HEAD_DIM, :], pd[:HEAD_DIM, :])
                    nc.sync.dma_start(rec2[HEAD_DIM:, :], rec[:HEAD_DIM, :])
                    nc.vector.tensor_mul(
                        outs[HEAD_DIM:, e, bs], pd[HEAD_DIM:, :], rec2[HEAD_DIM:, :]
                    )
                    if debug:
                        it = 4 * blk + 2 * e + hpar
                        nc.sync.dma_start(
                            drec.rearrange("p (i t) -> p i t", i=8)[:, it, :],
                            rec2[HEAD_DIM:, :],
                        )

            # ---------------- emission schedule
            emit_kv_chunk(0)
            emit_q_chunk(0, 0)
            emit_q_chunk(1, 0)
            for t in range(4):
                emit_v_tile(t)
            emit_kv_chunk(1)
            for t in range(4, 8):
                emit_v_tile(t)
            emit_kv_chunk(2)
            for t in range(8, 12):
                emit_v_tile(t)
            emit_kv_chunk(3)
            for t in range(12, 16):
                emit_v_tile(t)
            emit_q_chunk(0, 1)
            emit_q_chunk(1, 1)

            # BCD iterations: blk0 heads 0..3 (+ stage-A tail), E(b0), blk1, E(b1)
            emit_iteration(0, 0, 0, extra={5: lambda: emit_q_chunk(2, 0)})
            emit_iteration(0, 0, 1, extra={5: lambda: emit_q_chunk(3, 0)})
            emit_iteration(0, 1, 0, extra={5: lambda: emit_q_chunk(2, 1)})
            emit_iteration(0, 1, 1, extra={5: lambda: emit_q_chunk(3, 1)})
            emit_out_chunk(0)
            emit_out_chunk(1)
            emit_iteration(1, 0, 0)
            emit_iteration(1, 0, 1)
            emit_iteration(1, 1, 0)
            emit_iteration(1, 1, 1)
            emit_out_chunk(2)
            emit_out_chunk(3)

            if debug:
                dqt = nc.declare_dram_parameter("dbg_qt", [P, ET * TQ], F16, isOutput=True)
                dkv = nc.declare_dram_parameter("dbg_kv", [P, TKV], F16, isOutput=True)
                dk2 = nc.declare_dram_parameter("dbg_k2", [P, TKV], F16, isOutput=True)
                douts = nc.declare_dram_parameter("dbg_outs", [P, ET * TQ], F16, isOutput=True)
                dvp = nc.declare_dram_parameter("dbg_vp16A", [P, NTK * P], F16, isOutput=True)
                nc.sync.dma_start(dqt.rearrange("p (e t) -> p e t", e=ET), qt[:, :, :])
                nc.sync.dma_start(dkv[:, :], kv[:, :])
                nc.sync.dma_start(dk2[:, :], kc[:, :])
                nc.sync.dma_start(douts.rearrange("p (e t) -> p e t", e=ET), outs[:, :, :])
                nc.sync.dma_start(dvp.rearrange("p (n m) -> p n m", n=NTK), vp16A[:, :, :])

    nc.finalize()
    return nc


_NC_CACHE = None


def _get_nc():
    global _NC_CACHE
    if _NC_CACHE is None:
        # fp8 P@V and the DVE bit-trick exp are numerically too coarse for
        # this harness' max-error metric (peaked softmax columns expose P's
        # relative quantization directly): measured 1.8-2.9e-2 vs the 2e-2
        # gate.  True f16 exp on ACT + f16 P@V measures 8.6e-4.
        _NC_CACHE = build_bass(use_fp8=False, use_bitexp=False)
    return _NC_CACHE


def _cid():
    c = np.zeros((P, P + 64), dtype=np.float16)
    c[:, :P] = np.eye(P, dtype=np.float32)
    c[:, P:] = 1.0
    return c


def shard_inputs(query, context, Wq, Wk, Wv, Wo):
    """host-side sharding: 8 cores = batch(2) x kv-group(4)"""
    in_maps = []
    xqT = [np.ascontiguousarray(query[b].T).astype(np.float16) for b in range(B)]
    xcT = [np.ascontiguousarray(context[b].T).astype(np.float16) for b in range(B)]
    for core in range(N_CORES):
        b, g = divmod(core, GROUPS)
        wqT = np.ascontiguousarray(Wq[g * DQ : (g + 1) * DQ, :].T).astype(np.float16)
        wkvT = np.ascontiguousarray(
            np.concatenate(
                [
                    Wk[g * HEAD_DIM : (g + 1) * HEAD_DIM, :],
                    Wv[g * HEAD_DIM : (g + 1) * HEAD_DIM, :],
                ],
                axis=0,
            ).T
        ).astype(np.float16)
        woT = np.ascontiguousarray(Wo[:, g * DQ : (g + 1) * DQ].T).astype(np.float16)
        in_maps.append(
            {
                "xqT": xqT[b],
                "xcT": xcT[b],
                "wqT": wqT,
                "wkvT": wkvT,
                "woT": woT,
                "cid": _cid(),
            }
        )
    return in_maps


def kernel(query, context, Wq, Wk, Wv, Wo, _want_profile=False):
    from concourse.bass_utils import run_bass_kernel_spmd

    nc = _get_nc()
    in_maps = shard_inputs(query, context, Wq, Wk, Wv, Wo)
    res = run_bass_kernel_spmd(
        nc, in_maps, core_ids=list(range(N_CORES)), trace=_want_profile
    )
    out = np.zeros((B, TQ, D_MODEL), dtype=np.float32)
    for core in range(N_CORES):
        b = core // GROUPS
        out[b] += res.results[core]["yT"].T
    if _want_profile:
        return out, res
    return out


# revision 33
# speedup vs baseline: 1.5693x; 1.0851x over previous
"""Cross-attention (GQA) Trainium2 Bass kernel, v3.

Problem: B=2, Tq=Tkv=2048, D_MODEL=1024, 16 query heads / 4 kv heads,
head_dim=64.  Sharded over 8 NeuronCores as batch(2) x kv-group(4); each
core computes 4 query heads + its single kv head and a partial output
projection (Wo row-split by head group); partials are summed on host.

Design (v3):
  * single-head (blk x head) iterations with a pipelined B->exp->D
    schedule; exp is true f16 exp on ACT (the DVE bit-trick exp and fp8
    P@V measured 1.8-2.9e-2 max-rel-err -- peaked softmax columns expose
    P's relative quantization directly -- vs the 2e-2 gate, so they're
    disabled).  Steady state is ACT-paced at ~1.0us/tile.
  * scores carry a constant offset row (stationary row 64 = C, moving
    row 64 = 1) kept from the bit-exp experiments; ACT's free bias
    compensates.  Numerically neutral, it keeps exp outputs in a safe
    f16 range for any score outliers.
  * softmax denominators from ones-columns in the P@V stationary;
    reciprocal_approx_fast (only on partitions 0-63: its custom-DVE
    uops are broken at nonzero partition offsets).
  * input DMAs use full-row 2KB descriptors (descriptor count, not
    bytes, bounds DMA: ~57ns/descriptor/queue), split in tq-halves
    across queues; DMA dispatches are spread over the sync, gpsimd and
    vector sequencers (each DIRECT2D dispatch costs ~0.6us serialized
    on its issuing engine's sequencer).
  * E (output projection) for blk0 is interleaved into blk1's
    iterations; output yT is f16 (host accumulates in f32).
  * PSUM: scores pool 2x2 banks, accumulator pool 2x2 banks; PE warmup
    ramps the DVFS p-state (1.2 -> 2.4 GHz) before the first projection.
"""

import os
import sys

import numpy as np

for _p in ("/opt/trn_rl_repo",):
    if _p not in sys.path and os.path.isdir(_p):
        sys.path.insert(0, _p)

import concourse.bass as bass
import concourse.bacc as bacc
import concourse.mybir as mybir
from concourse.tile import TileContext

# ---------------------------------------------------------------- problem dims
B = 2
TQ = 2048
TKV = 2048
D_MODEL = 1024
N_HEADS = 16
N_KV_HEADS = 4
HEAD_DIM = 64
N_CORES = 8
GROUPS = N_KV_HEADS  # kv groups = 4
HEADS_PER_DEV = N_HEADS // GROUPS  # 4
DQ = HEADS_PER_DEV * HEAD_DIM  # 256
DKV = 2 * HEAD_DIM  # 128 (k rows + v rows stacked)
SCALE = 1.0 / float(np.sqrt(HEAD_DIM))

P = 128
FREE = 512  # matmul moving-operand chunk / psum bank width
BLK = 1024  # tq block width

F32 = mybir.dt.float32
F16 = mybir.dt.float16

DT = D_MODEL // P  # 8 d-tiles
ET = DQ // P  # 2 e-tiles (query head pairs)
NCH = TQ // FREE  # 4 chunks of 512
NTK = TKV // P  # 16 tk tiles
NBLK = TQ // BLK  # 2 tq blocks
JPB = BLK // FREE  # 2 free-chunks per block
MT = D_MODEL // P  # 8 output m-tiles

# exp scale handling: scores carry +C_DEV via an offset row; ACT's bias
# restores e^(s/8 - ln2*(15 - AEXP*C/1024)) -- a benign global scale that
# cancels in the softmax normalization.
SHIFT = 3.75
AEXP = 1024.0 * 1.4426950408889634 / 8.0
_LOG2E = 1.4426950408889634
C_OFFSET = (15360.0 - 44.5 - SHIFT * 1024.0 * _LOG2E) / AEXP
C_DEV = float(np.float16(C_OFFSET))
ACT_BIAS = float(np.log(2.0) * (AEXP * C_DEV / 1024.0 - 15.0) - C_DEV / 8.0)


def build_bass():
    nc = bacc.Bacc()

    xq = nc.declare_dram_parameter("xqT", [D_MODEL, TQ], F16, isOutput=False)
    xc = nc.declare_dram_parameter("xcT", [D_MODEL, TKV], F16, isOutput=False)
    wq = nc.declare_dram_parameter("wqT", [D_MODEL, DQ], F16, isOutput=False)
    wkv = nc.declare_dram_parameter("wkvT", [D_MODEL, DKV], F16, isOutput=False)
    wo = nc.declare_dram_parameter("woT", [DQ, D_MODEL], F16, isOutput=False)
    cid = nc.declare_dram_parameter("cid", [P, P + 64], F16, isOutput=False)
    yt = nc.declare_dram_parameter("yT", [D_MODEL, TQ], F16, isOutput=True)

    with TileContext(nc) as tc:
        with (
            tc.tile_pool(name="consts", bufs=1) as consts,
            tc.tile_pool(name="pt", bufs=3) as ptpool,
            tc.tile_pool(name="nrm", bufs=2) as nrmpool,
            tc.tile_pool(name="yout", bufs=3) as ypool,
            tc.tile_pool(name="psB", bufs=2, space="PSUM") as psB,
            tc.tile_pool(name="psD", bufs=2, space="PSUM") as psD,
        ):
            # ---------------- constants
            ident = consts.tile([P, P + 64], F16, tag="ident")
            nc.sync.dma_start(ident, cid[:])
            wq_sb = consts.tile([P, DT, DQ], F16, tag="wq")
            nc.sync.dma_start(wq_sb, wq.rearrange("(i p) e -> p i e", p=P))
            wkv_sb = consts.tile([P, DT, DKV], F16, tag="wkv")
            nc.sync.dma_start(wkv_sb, wkv.rearrange("(i p) e -> p i e", p=P))
            wo_sb = consts.tile([P, ET, D_MODEL], F16, tag="wo")
            nc.sync.dma_start(wo_sb, wo.rearrange("(i p) m -> p i m", p=P))

            # input staging (full width; loaded in tq-halves per d-tile so
            # each DMA call is 128 rows x 2KB descriptors on its own queue)
            xc_t = consts.tile([P, DT, TKV], F16, tag="xc")
            xq_t = consts.tile([P, DT, TQ], F16, tag="xq")
            xc_r = xc.rearrange("(i p) t -> p i t", p=P)
            xq_r = xq.rearrange("(i p) t -> p i t", p=P)
            HALF = TQ // 2
            for i in range(DT):  # wave 1: first halves
                nc.sync.dma_start(xc_t[:, i, :HALF], xc_r[:, i, :HALF])
                nc.gpsimd.dma_start(xq_t[:, i, :HALF], xq_r[:, i, :HALF])
            for i in range(DT):  # wave 2: second halves
                nc.sync.dma_start(xc_t[:, i, HALF:], xc_r[:, i, HALF:])
                nc.gpsimd.dma_start(xq_t[:, i, HALF:], xq_r[:, i, HALF:])

            qt = consts.tile([P, ET, TQ], F16, tag="qt")  # proj staging
            kv = consts.tile([P, TKV], F16, tag="kv")  # rows 0-63 kT, 64-127 vT
            kc = consts.tile([P, TKV], F16, tag="kc")  # [kT; C-row]
            qt65 = consts.tile([P, HEADS_PER_DEV, TQ], F16, tag="qt65")  # [q_h; 1]
            vp16A = consts.tile([P, NTK, P], F16, tag="vp16A")  # [v | ones]
            vp16B = consts.tile([P, NTK, P], F16, tag="vp16B")  # [ones | v]
            outs = consts.tile([P, ET, TQ], F16, tag="outs")  # normalized outT

            nc.vector.memset(vp16A[:, :, HEAD_DIM:], 1.0)
            nc.vector.memset(vp16B[:, :, :HEAD_DIM], 1.0)
            nc.vector.memset(kc[HEAD_DIM : HEAD_DIM + 1, :], C_DEV)
            nc.vector.memset(qt65[HEAD_DIM : HEAD_DIM + 1, :, :], 1.0)
            bshift = consts.tile([P, 1], F32, tag="bshift")
            nc.vector.memset(bshift, ACT_BIAS)

            # ---------------- PE warmup: ramp the DVFS p-state
            for w in range(5):
                pwarm = psB.tile([P, P + 64], F32, tag="ps", name=f"warm{w}")
                for i in range(8):
                    nc.tensor.matmul(
                        pwarm,
                        ident[:, :P],
                        ident[:],
                        start=(i == 0),
                        stop=(i == 7),
                    )

            # ---------------- stage A emitters
            def emit_kv_chunk(c):
                cs = slice(c * FREE, (c + 1) * FREE)
                pkv = psB.tile([P, FREE], F32, tag="ps", name=f"pkv{c}")
                for i in range(DT):
                    nc.tensor.matmul(
                        pkv,
                        wkv_sb[:, i, :],
                        xc_t[:, i, cs],
                        start=(i == 0),
                        stop=(i == DT - 1),
                    )
                nc.vector.tensor_copy(kv[:, cs], pkv)
                h2 = FREE // 2
                for s in range(2):
                    ss = slice(c * FREE + s * h2, c * FREE + (s + 1) * h2)
                    nc.gpsimd.dma_start(kc[:HEAD_DIM, ss], kv[:HEAD_DIM, ss])

            def emit_q_chunk(c, e):
                cs = slice(c * FREE, (c + 1) * FREE)
                pq = psB.tile([P, FREE], F32, tag="ps", name=f"pq{c}_{e}")
                for i in range(DT):
                    nc.tensor.matmul(
                        pq,
                        wq_sb[:, i, e * P : (e + 1) * P],
                        xq_t[:, i, cs],
                        start=(i == 0),
                        stop=(i == DT - 1),
                    )
                nc.vector.tensor_copy(qt[:, e, cs], pq)
                # shift into per-head layout (DMA can cross partitions)
                h2 = FREE // 2
                for s in range(2):
                    ss = slice(c * FREE + s * h2, c * FREE + (s + 1) * h2)
                    nc.gpsimd.dma_start(
                        qt65[:HEAD_DIM, 2 * e, ss], qt[:HEAD_DIM, e, ss]
                    )
                    nc.gpsimd.dma_start(
                        qt65[:HEAD_DIM, 2 * e + 1, ss], qt[HEAD_DIM:, e, ss]
                    )

            def emit_v_tiles(c):
                for t in range(4 * c, 4 * c + 4):
                    ts_ = slice(t * P, (t + 1) * P)
                    pv = psB.tile([P, HEAD_DIM], F16, tag="ps", name=f"pv{t}")
                    nc.tensor.transpose(
                        pv,
                        kv[HEAD_DIM : 2 * HEAD_DIM, ts_],
                        ident[HEAD_DIM:, HEAD_DIM:P],
                    )
                    nc.vector.tensor_copy(vp16A[:, t, :HEAD_DIM], pv)
                    nc.vector.tensor_copy(vp16B[:, t, HEAD_DIM:], pv)

            # ---------------- stage E: one (chunk, m) unit
            def emit_out_unit(c, m):
                cs = slice(c * FREE, (c + 1) * FREE)
                ms = slice(m * P, (m + 1) * P)
                py = psB.tile([P, FREE], F32, tag="ps", name=f"py{c}_{m}")
                for ee in range(ET):
                    nc.tensor.matmul(
                        py,
                        wo_sb[:, ee, ms],
                        outs[:, ee, cs],
                        start=(ee == 0),
                        stop=(ee == ET - 1),
                    )
                yo = ypool.tile([P, FREE], F16, tag="yout", name=f"yo{c}_{m}")
                nc.vector.tensor_copy(yo, py)
                nc.gpsimd.dma_start(yt[ms, cs], yo)

            # ---------------- one (blk, head) iteration of B -> exp -> D
            def emit_iteration(blk, e, hpar, extra=None):
                bs = slice(blk * BLK, (blk + 1) * BLK)
                head = 2 * e + hpar
                vp16 = vp16A if hpar == 0 else vp16B

                pd = psD.tile([P, BLK], F32, tag="pd", name=f"pd{blk}_{e}_{hpar}")
                pts = {}

                def emit_d(t, first, last):
                    src = pts.pop(t)
                    for j in range(JPB):
                        jo = slice(j * FREE, (j + 1) * FREE)
                        nc.tensor.matmul(
                            pd[:, jo],
                            vp16[:, t, :],
                            src[:, jo],
                            start=first,
                            stop=last,
                            skip_group_check=True,
                        )

                for t in range(NTK):
                    ts_ = slice(t * P, (t + 1) * P)
                    pb = psB.tile([P, BLK], F32, tag="ps", name=f"pb{t}")
                    for j in range(JPB):
                        js = slice(blk * BLK + j * FREE, blk * BLK + (j + 1) * FREE)
                        jo = slice(j * FREE, (j + 1) * FREE)
                        nc.tensor.matmul(
                            pb[:, jo],
                            kc[: HEAD_DIM + 1, ts_],
                            qt65[: HEAD_DIM + 1, head, js],
                        )
                    pts[t] = ptpool.tile([P, BLK], F16, tag="pt", name=f"pt{t}")
                    nc.scalar.activation(
                        pts[t],
                        pb,
                        mybir.ActivationFunctionType.Exp,
                        bias=bshift[:, :],
                        scale=SCALE,
                    )
                    # D for tile t-2 (software-pipelined behind exp)
                    if t >= 2:
                        emit_d(t - 2, first=(t == 2), last=False)
                    if extra and t in extra:
                        extra[t]()
                emit_d(NTK - 2, first=False, last=False)
                emit_d(NTK - 1, first=False, last=True)

                # normalize: out = num * approx(1/denom).
                # reciprocal_approx_fast only works at partition base 0, so
                # for even heads (denominator on PSUM rows 64-127) shift the
                # denominator down via SBUF-SBUF DMA first.
                rec = nrmpool.tile([P, BLK], F32, tag="rec")
                if hpar == 0:
                    dnm = nrmpool.tile([P, BLK], F32, tag="dnm")
                    dnm2 = nrmpool.tile([P, BLK], F32, tag="dnm2")
                    nc.vector.tensor_copy(dnm[HEAD_DIM:, :], pd[HEAD_DIM:, :])
                    nc.gpsimd.dma_start(dnm2[:HEAD_DIM, :], dnm[HEAD_DIM:, :])
                    nc.vector.reciprocal_approx_fast(
                        rec[:HEAD_DIM, :], dnm2[:HEAD_DIM, :]
                    )
                    nc.vector.tensor_mul(
                        outs[:HEAD_DIM, e, bs], pd[:HEAD_DIM, :], rec[:HEAD_DIM, :]
                    )
                else:
                    rec2 = nrmpool.tile([P, BLK], F32, tag="rec2")
                    nc.vector.reciprocal_approx_fast(rec[:HEAD_DIM, :], pd[:HEAD_DIM, :])
                    nc.gpsimd.dma_start(rec2[HEAD_DIM:, :], rec[:HEAD_DIM, :])
                    nc.vector.tensor_mul(
                        outs[HEAD_DIM:, e, bs], pd[HEAD_DIM:, :], rec2[HEAD_DIM:, :]
                    )

            # ---------------- emission schedule
            emit_kv_chunk(0)
            emit_kv_chunk(1)
            emit_q_chunk(0, 0)
            emit_q_chunk(1, 0)
            emit_v_tiles(0)
            emit_v_tiles(1)

            emit_iteration(
                0, 0, 0,
                extra={
                    2: lambda: emit_q_chunk(0, 1),
                    4: lambda: emit_q_chunk(1, 1),
                    6: lambda: emit_kv_chunk(2),
                    7: lambda: emit_v_tiles(2),
                    8: lambda: emit_kv_chunk(3),
                    9: lambda: emit_v_tiles(3),
                },
            )
            emit_iteration(
                0, 0, 1,
                extra={3: lambda: emit_q_chunk(2, 0), 6: lambda: emit_q_chunk(3, 0)},
            )
            emit_iteration(
                0, 1, 0,
                extra={3: lambda: emit_q_chunk(2, 1), 6: lambda: emit_q_chunk(3, 1)},
            )
            emit_iteration(0, 1, 1)
            # blk1 iterations carry blk0's output projection
            emit_iteration(
                1, 0, 0,
                extra={2 * m + 1: (lambda m=m: emit_out_unit(0, m)) for m in range(MT)},
            )
            emit_iteration(
                1, 0, 1,
                extra={2 * m + 1: (lambda m=m: emit_out_unit(1, m)) for m in range(MT)},
            )
            emit_iteration(1, 1, 0)
            emit_iteration(1, 1, 1)
            for m in range(MT):
                emit_out_unit(2, m)
            for m in range(MT):
                emit_out_unit(3, m)

    nc.finalize()
    return nc


_NC_CACHE = None


def _get_nc():
    global _NC_CACHE
    if _NC_CACHE is None:
        _NC_CACHE = build_bass()
    return _NC_CACHE


def _cid():
    c = np.zeros((P, P + 64), dtype=np.float16)
    c[:, :P] = np.eye(P, dtype=np.float32)
    c[:, P:] = 1.0
    return c


def shard_inputs(query, context, Wq, Wk, Wv, Wo):
    """host-side sharding: 8 cores = batch(2) x kv-group(4)"""
    in_maps = []
    xqT = [np.ascontiguousarray(query[b].T).astype(np.float16) for b in range(B)]
    xcT = [np.ascontiguousarray(context[b].T).astype(np.float16) for b in range(B)]
    for core in range(N_CORES):
        b, g = divmod(core, GROUPS)
        wqT = np.ascontiguousarray(Wq[g * DQ : (g + 1) * DQ, :].T).astype(np.float16)
        wkvT = np.ascontiguousarray(
            np.concatenate(
                [
                    Wk[g * HEAD_DIM : (g + 1) * HEAD_DIM, :],
                    Wv[g * HEAD_DIM : (g + 1) * HEAD_DIM, :],
                ],
                axis=0,
            ).T
        ).astype(np.float16)
        woT = np.ascontiguousarray(Wo[:, g * DQ : (g + 1) * DQ].T).astype(np.float16)
        in_maps.append(
            {
                "xqT": xqT[b],
                "xcT": xcT[b],
                "wqT": wqT,
                "wkvT": wkvT,
                "woT": woT,
                "cid": _cid(),
            }
        )
    return in_maps


def kernel(query, context, Wq, Wk, Wv, Wo, _want_profile=False):
    from concourse.bass_utils import run_bass_kernel_spmd

    nc = _get_nc()
    in_maps = shard_inputs(query, context, Wq, Wk, Wv, Wo)
    res = run_bass_kernel_spmd(
        nc, in_maps, core_ids=list(range(N_CORES)), trace=_want_profile
    )
    out = np.zeros((B, TQ, D_MODEL), dtype=np.float32)
    for core in range(N_CORES):
        b = core // GROUPS
        out[b] += res.results[core]["yT"].T.astype(np.float32)
    if _want_profile:
        return out, res
    return out
